# revision 19
# baseline (speedup 1.0000x reference)
"""CenterLoss kernel for Trainium2, data-parallel over 8 NeuronCores.

loss = 0.5 * mean_b ||hidden[b] - centers[y[b]]||^2

Per core: a 128-row batch shard. The [B, C] distance matrix of the reference
is never materialized -- only the true-class center row per sample is needed,
fetched with an indirect-DMA gather. The subtraction is fused into the gather
itself: the tile is pre-loaded with hidden, and the gather of (host-negated)
centers uses the DMA engine's inline CCE add, so compute is a single DVE
tensor_tensor_reduce (square + row-sum) per core. Host sums the per-row
partials across cores (the "all-reduce" of the scalar).
"""

import os

import numpy as np

from concourse import bass, bass_isa, mybir
import concourse.tile as tile
from concourse.bass_utils import run_bass_kernel_spmd

N_CORES = 8
B, C, D = 1024, 1000, 512
S = B // N_CORES  # 128 rows per core
SCALE = 0.5 / B  # 1/2048, exact power of two -> lossless f32 scaling

F32 = mybir.dt.float32

# "raw":   raw-bass minimal-sync version of "fused" (no Tile barriers)
# "fused": Tile, gather-with-CCE-add of negated centers (1 DVE op total)
# "plain": Tile, gather + copy-chain + sub + TTR (fallback, no CCE compute)
VARIANT = os.environ.get("CENTERLOSS_VARIANT", "raw2")


def _build_raw():
    nc = bass.Bass()
    y_t = nc.dram_tensor("y_idx", [S, 1], mybir.dt.int32, kind="ExternalInput")
    h_t = nc.dram_tensor("hidden_shard", [S, D], F32, kind="ExternalInput")
    c_t = nc.dram_tensor("neg_centers", [C, D], F32, kind="ExternalInput")
    o_t = nc.dram_tensor("partial", [S, 1], F32, kind="ExternalOutput")

    with (
        nc.sbuf_tensor([S, 1], mybir.dt.int32) as idx,
        nc.sbuf_tensor([S, D], F32) as t,
        nc.sbuf_tensor([S, D], F32) as sq,
        nc.sbuf_tensor([S, 1], F32) as part,
        nc.semaphore("sem_idx") as sem_idx,
        nc.semaphore("sem_in") as sem_in,
        nc.semaphore("sem_g") as sem_g,
        nc.semaphore("sem_dve") as sem_dve,
        nc.semaphore("sem_out") as sem_out,
        nc.Block() as block,
    ):


        @block.sync
        def _(sync):
            sync.dma_start(out=idx[:], in_=y_t[:]).then_inc(sem_idx, 16)
            sync.dma_start(out=t[:], in_=h_t[:]).then_inc(sem_in, 16)
            sync.wait_ge(sem_dve, 1)
            sync.dma_start(out=o_t[:], in_=part[:, :1]).then_inc(sem_out, 16)
            sync.wait_ge(sem_out, 16)

        @block.gpsimd
        def _(g):
            g.wait_ge(sem_idx, 16)
            g.wait_ge(sem_in, 16)
            # t := (-centers[y]) + t  (inline CCE add during the gather)
            g.indirect_dma_start(
                out=t[:],
                out_offset=None,
                in_=c_t[:],
                in_offset=bass.IndirectOffsetOnAxis(ap=idx[:, :1], axis=0),
                compute_op=mybir.AluOpType.add,
            ).then_inc(sem_g, 16)

        @block.vector
        def _(v):
            # gather completion implies h already landed in t (gpsimd waited)
            v.wait_ge(sem_g, 16)
            # part[p] = sum_d (t[p,d]*SCALE) * t[p,d]  -- square+scale+row-sum
            v.scalar_tensor_tensor(
                out=sq[:],
                in0=t[:],
                scalar=SCALE,
                in1=t[:],
                op0=mybir.AluOpType.mult,
                op1=mybir.AluOpType.mult,
                accum_out=part[:, :1],
            ).then_inc(sem_dve, 1)

        # Epilogue (mirrors Tile's): barrier across the three active engines,
        # then Pool clears every semaphore so the NEFF is re-executable.
        nc.multi_engine_barrier(
            [mybir.EngineType.Pool, mybir.EngineType.DVE, mybir.EngineType.SP]
        )
        for s in (sem_idx, sem_in, sem_g, sem_dve, sem_out):
            nc.gpsimd.sem_clear(s)

    return nc


def _build_raw2():
    """Unfused: gather gated only on idx; h joins at the DVE instead, so the
    h-DMA completion is off the gather's critical path."""
    nc = bass.Bass()
    y_t = nc.dram_tensor("y_idx", [S, 1], mybir.dt.int32, kind="ExternalInput")
    h_t = nc.dram_tensor("hidden_shard", [S, D], F32, kind="ExternalInput")
    c_t = nc.dram_tensor("neg_centers", [C, D], F32, kind="ExternalInput")
    o_t = nc.dram_tensor("partial", [S, 1], F32, kind="ExternalOutput")

    with (
        nc.sbuf_tensor([S, 1], mybir.dt.int32) as idx,
        nc.sbuf_tensor([S, D], F32) as h,
        nc.sbuf_tensor([S, D], F32) as cg,
        nc.sbuf_tensor([S, D], F32) as d,
        nc.sbuf_tensor([S, D], F32) as sq,
        nc.sbuf_tensor([S, 1], F32) as part,
        nc.semaphore("sem_idx") as sem_idx,
        nc.semaphore("sem_in") as sem_in,
        nc.semaphore("sem_g") as sem_g,
        nc.semaphore("sem_d") as sem_d,
        nc.semaphore("sem_dve") as sem_dve,
        nc.semaphore("sem_out") as sem_out,
        nc.Block() as block,
    ):

        @block.sync
        def _(sync):
            sync.dma_start(out=idx[:], in_=y_t[:]).then_inc(sem_idx, 16)
            sync.wait_ge(sem_dve, 1)
            sync.dma_start(out=o_t[:], in_=part[:, :1]).then_inc(sem_out, 16)
            sync.wait_ge(sem_out, 16)

        @block.scalar
        def _(sc):
            # h load on ACT's HWDGE ring: overlaps SP's idx dispatch
            sc.dma_start(out=h[:], in_=h_t[:]).then_inc(sem_in, 16)

        @block.gpsimd
        def _(g):
            g.wait_ge(sem_idx, 16)
            g.indirect_dma_start(
                out=cg[:],
                out_offset=None,
                in_=c_t[:],
                in_offset=bass.IndirectOffsetOnAxis(ap=idx[:, :1], axis=0),
            ).then_inc(sem_g, 16)

        @block.vector
        def _(v):
            v.wait_ge(sem_g, 16)
            v.wait_ge(sem_in, 16)
            # d = h + (-c); then part[p] = sum_d (d*SCALE)*d
            # (sem between the two: DVE is pipelined, same-engine RAW needs it)
            v.tensor_add(out=d[:], in0=h[:], in1=cg[:]).then_inc(sem_d, 1)
            v.wait_ge(sem_d, 1)
            v.scalar_tensor_tensor(
                out=sq[:],
                in0=d[:],
                scalar=SCALE,
                in1=d[:],
                op0=mybir.AluOpType.mult,
                op1=mybir.AluOpType.mult,
                accum_out=part[:, :1],
            ).then_inc(sem_dve, 1)

        nc.multi_engine_barrier(
            [
                mybir.EngineType.Pool,
                mybir.EngineType.Activation,
                mybir.EngineType.DVE,
                mybir.EngineType.SP,
            ]
        )
        for s in (sem_idx, sem_in, sem_g, sem_d, sem_dve, sem_out):
            nc.gpsimd.sem_clear(s)

    return nc


def _build_raw3():
    """raw2 + the gather reads its offsets directly from DRAM: no idx DMA,
    no wait before the gather at all."""
    nc = bass.Bass()
    y_t = nc.dram_tensor("y_idx", [S, 1], mybir.dt.int32, kind="ExternalInput")
    h_t = nc.dram_tensor("hidden_shard", [S, D], F32, kind="ExternalInput")
    c_t = nc.dram_tensor("neg_centers", [C, D], F32, kind="ExternalInput")
    o_t = nc.dram_tensor("partial", [S, 1], F32, kind="ExternalOutput")

    with (
        nc.sbuf_tensor([S, D], F32) as h,
        nc.sbuf_tensor([S, D], F32) as cg,
        nc.sbuf_tensor([S, D], F32) as d,
        nc.sbuf_tensor([S, D], F32) as sq,
        nc.sbuf_tensor([S, 1], F32) as part,
        nc.semaphore("sem_in") as sem_in,
        nc.semaphore("sem_g") as sem_g,
        nc.semaphore("sem_d") as sem_d,
        nc.semaphore("sem_dve") as sem_dve,
        nc.semaphore("sem_out") as sem_out,
        nc.Block() as block,
    ):

        @block.sync
        def _(sync):
            sync.wait_ge(sem_dve, 1)
            sync.dma_start(out=o_t[:], in_=part[:, :1]).then_inc(sem_out, 16)
            sync.wait_ge(sem_out, 16)

        @block.scalar
        def _(sc):
            sc.dma_start(out=h[:], in_=h_t[:]).then_inc(sem_in, 16)

        @block.gpsimd
        def _(g):
            g.indirect_dma_start(
                out=cg[:],
                out_offset=None,
                in_=c_t[:],
                in_offset=bass.IndirectOffsetOnAxis(ap=y_t[:, :1], axis=0),
            ).then_inc(sem_g, 16)

        @block.vector
        def _(v):
            v.wait_ge(sem_g, 16)
            v.wait_ge(sem_in, 16)
            v.tensor_add(out=d[:], in0=h[:], in1=cg[:]).then_inc(sem_d, 1)
            v.wait_ge(sem_d, 1)
            v.scalar_tensor_tensor(
                out=sq[:],
                in0=d[:],
                scalar=SCALE,
                in1=d[:],
                op0=mybir.AluOpType.mult,
                op1=mybir.AluOpType.mult,
                accum_out=part[:, :1],
            ).then_inc(sem_dve, 1)

        nc.multi_engine_barrier(
            [
                mybir.EngineType.Pool,
                mybir.EngineType.Activation,
                mybir.EngineType.DVE,
                mybir.EngineType.SP,
            ]
        )
        for s in (sem_in, sem_g, sem_d, sem_dve, sem_out):
            nc.gpsimd.sem_clear(s)

    return nc


def _build_raw4(n_split=2):
    """raw2 + gather split into row groups: the DVE's add on group k overlaps
    the transfer of group k+1, and the per-DMA completion latencies overlap."""
    nc = bass.Bass()
    y_t = nc.dram_tensor("y_idx", [S, 1], mybir.dt.int32, kind="ExternalInput")
    h_t = nc.dram_tensor("hidden_shard", [S, D], F32, kind="ExternalInput")
    c_t = nc.dram_tensor("neg_centers", [C, D], F32, kind="ExternalInput")
    o_t = nc.dram_tensor("partial", [S, 1], F32, kind="ExternalOutput")

    R = S // n_split  # rows per gather group

    with (
        nc.sbuf_tensor([S, 1], mybir.dt.int32) as idx,
        nc.sbuf_tensor([S, D], F32) as h,
        nc.sbuf_tensor([S, D], F32) as cg,
        nc.sbuf_tensor([S, D], F32) as d,
        nc.sbuf_tensor([S, D], F32) as sq,
        nc.sbuf_tensor([S, 1], F32) as part,
        nc.semaphore("sem_idx") as sem_idx,
        nc.semaphore("sem_in") as sem_in,
        nc.semaphore("sem_g") as sem_g,
        nc.semaphore("sem_d") as sem_d,
        nc.semaphore("sem_dve") as sem_dve,
        nc.semaphore("sem_out") as sem_out,
        nc.Block() as block,
    ):

        @block.sync
        def _(sync):
            sync.dma_start(out=idx[:], in_=y_t[:]).then_inc(sem_idx, 16)
            sync.wait_ge(sem_dve, 1)
            sync.dma_start(out=o_t[:], in_=part[:, :1]).then_inc(sem_out, 16)
            sync.wait_ge(sem_out, 16)

        @block.scalar
        def _(sc):
            sc.dma_start(out=h[:], in_=h_t[:]).then_inc(sem_in, 16)

        @block.gpsimd
        def _(g):
            g.wait_ge(sem_idx, 16)
            for k in range(n_split):
                r0, r1 = k * R, (k + 1) * R
                g.indirect_dma_start(
                    out=cg[r0:r1, :],
                    out_offset=None,
                    in_=c_t[:],
                    in_offset=bass.IndirectOffsetOnAxis(ap=idx[r0:r1, :1], axis=0),
                ).then_inc(sem_g, 16)

        @block.vector
        def _(v):
            v.wait_ge(sem_in, 16)
            for k in range(n_split):
                r0, r1 = k * R, (k + 1) * R
                v.wait_ge(sem_g, 16 * (k + 1))
                add = v.tensor_add(
                    out=d[r0:r1, :], in0=h[r0:r1, :], in1=cg[r0:r1, :]
                )
            # completion-sem on the last add drains the pipelined adds before
            # the same-engine RAW read of d (in-order retire covers the rest)
            add.then_inc(sem_d, 1)
            v.wait_ge(sem_d, 1)
            v.scalar_tensor_tensor(
                out=sq[:],
                in0=d[:],
                scalar=SCALE,
                in1=d[:],
                op0=mybir.AluOpType.mult,
                op1=mybir.AluOpType.mult,
                accum_out=part[:, :1],
            ).then_inc(sem_dve, 1)

        nc.multi_engine_barrier(
            [
                mybir.EngineType.Pool,
                mybir.EngineType.Activation,
                mybir.EngineType.DVE,
                mybir.EngineType.SP,
            ]
        )
        for s in (sem_idx, sem_in, sem_g, sem_d, sem_dve, sem_out):
            nc.gpsimd.sem_clear(s)

    return nc


def _build_raw5():
    """raw2 with idx in a single partition [1,S]: one-descriptor idx DMA,
    offsets read contiguously from partition 0."""
    nc = bass.Bass()
    y_t = nc.dram_tensor("y_idx", [1, S], mybir.dt.int32, kind="ExternalInput")
    h_t = nc.dram_tensor("hidden_shard", [S, D], F32, kind="ExternalInput")
    c_t = nc.dram_tensor("neg_centers", [C, D], F32, kind="ExternalInput")
    o_t = nc.dram_tensor("partial", [S, 1], F32, kind="ExternalOutput")

    with (
        nc.sbuf_tensor([1, S], mybir.dt.int32) as idx,
        nc.sbuf_tensor([S, D], F32) as h,
        nc.sbuf_tensor([S, D], F32) as cg,
        nc.sbuf_tensor([S, D], F32) as d,
        nc.sbuf_tensor([S, D], F32) as sq,
        nc.sbuf_tensor([S, 1], F32) as part,
        nc.semaphore("sem_idx") as sem_idx,
        nc.semaphore("sem_in") as sem_in,
        nc.semaphore("sem_g") as sem_g,
        nc.semaphore("sem_d") as sem_d,
        nc.semaphore("sem_dve") as sem_dve,
        nc.semaphore("sem_out") as sem_out,
        nc.Block() as block,
    ):

        @block.sync
        def _(sync):
            sync.dma_start(out=idx[:], in_=y_t[:]).then_inc(sem_idx, 16)
            sync.wait_ge(sem_dve, 1)
            sync.dma_start(out=o_t[:], in_=part[:, :1]).then_inc(sem_out, 16)
            sync.wait_ge(sem_out, 16)

        @block.scalar
        def _(sc):
            sc.dma_start(out=h[:], in_=h_t[:]).then_inc(sem_in, 16)

        @block.gpsimd
        def _(g):
            g.wait_ge(sem_idx, 16)
            g.indirect_dma_start(
                out=cg[:],
                out_offset=None,
                in_=c_t[:],
                in_offset=bass.IndirectOffsetOnAxis(ap=idx[:1, :S], axis=0),
            ).then_inc(sem_g, 16)

        @block.vector
        def _(v):
            v.wait_ge(sem_g, 16)
            v.wait_ge(sem_in, 16)
            v.tensor_add(out=d[:], in0=h[:], in1=cg[:]).then_inc(sem_d, 1)
            v.wait_ge(sem_d, 1)
            v.scalar_tensor_tensor(
                out=sq[:],
                in0=d[:],
                scalar=SCALE,
                in1=d[:],
                op0=mybir.AluOpType.mult,
                op1=mybir.AluOpType.mult,
                accum_out=part[:, :1],
            ).then_inc(sem_dve, 1)

        nc.multi_engine_barrier(
            [
                mybir.EngineType.Pool,
                mybir.EngineType.Activation,
                mybir.EngineType.DVE,
                mybir.EngineType.SP,
            ]
        )
        for s in (sem_idx, sem_in, sem_g, sem_d, sem_dve, sem_out):
            nc.gpsimd.sem_clear(s)

    return nc


F16 = mybir.dt.float16


def _build_v6():
    """raw3 + fp16 data path: h and centers are host-converted to f16
    (halves both DMA transfers; TT-sub runs in the DVE 2x mode), the
    square+row-sum is one tensor_tensor_reduce with an f32 accumulator.
    Host applies the 0.5/B scale."""
    nc = bass.Bass()
    y_t = nc.dram_tensor("y_idx", [S, 1], mybir.dt.int32, kind="ExternalInput")
    h_t = nc.dram_tensor("h16", [S, D], F16, kind="ExternalInput")
    c_t = nc.dram_tensor("c16", [C, D], F16, kind="ExternalInput")
    o_t = nc.dram_tensor("partial", [S, 1], F32, kind="ExternalOutput")

    with (
        nc.sbuf_tensor([S, D], F16) as h,
        nc.sbuf_tensor([S, D], F16) as cg,
        nc.sbuf_tensor([S, D], F16) as d,
        nc.sbuf_tensor([S, D], F16) as sq,
        nc.sbuf_tensor([S, 1], F32) as part,
        nc.semaphore("sem_h") as sem_h,
        nc.semaphore("sem_g") as sem_g,
        nc.semaphore("sem_d") as sem_d,
        nc.semaphore("sem_dve") as sem_dve,
        nc.semaphore("sem_out") as sem_out,
        nc.Block() as block,
    ):

        @block.sync
        def _(sync):
            sync.dma_start(out=h[:], in_=h_t[:]).then_inc(sem_h, 16)
            sync.wait_ge(sem_dve, 1)
            sync.dma_start(out=o_t[:], in_=part[:, :1]).then_inc(sem_out, 16)
            sync.wait_ge(sem_out, 16)

        @block.gpsimd
        def _(g):
            g.indirect_dma_start(
                out=cg[:],
                out_offset=None,
                in_=c_t[:],
                in_offset=bass.IndirectOffsetOnAxis(ap=y_t[:, :1], axis=0),
            ).then_inc(sem_g, 16)

        @block.vector
        def _(v):
            v.wait_ge(sem_g, 16)
            v.wait_ge(sem_h, 16)
            v.tensor_sub(out=d[:], in0=h[:], in1=cg[:]).then_inc(sem_d, 1)
            v.wait_ge(sem_d, 1)
            v.scalar_tensor_tensor(
                out=sq[:],
                in0=d[:],
                scalar=1.0,
                in1=d[:],
                op0=mybir.AluOpType.mult,
                op1=mybir.AluOpType.mult,
                accum_out=part[:, :1],
            ).then_inc(sem_dve, 1)

        nc.multi_engine_barrier(
            [mybir.EngineType.Pool, mybir.EngineType.DVE, mybir.EngineType.SP]
        )
        for s in (sem_h, sem_g, sem_d, sem_dve, sem_out):
            nc.gpsimd.sem_clear(s)

    return nc


def _build_v7():
    """v6 + the output leaves via a PREPARE_ONLY dma_scatter_add fired by
    trigger_dma: descriptor generation and the DGE->DMA launch latency move
    off the critical path (the plain-DMA HWDGE 625ns + 650ns launch are
    skipped; the trigger fires the pre-built descriptors directly).

    scatter-add needs a 256B-aligned DRAM row stride, so the output is
    [S, 64] f32 with the partial in column 0 (the rest is never written:
    scatter elem_size=1). Column 0 is pre-zeroed by a small early DMA so
    the CCE add lands on zero and the NEFF stays re-executable."""
    nc = bass.Bass()
    y_t = nc.dram_tensor("y_idx", [S, 1], mybir.dt.int32, kind="ExternalInput")
    h_t = nc.dram_tensor("h16", [S, D], F16, kind="ExternalInput")
    c_t = nc.dram_tensor("c16", [C, D], F16, kind="ExternalInput")
    o_t = nc.dram_tensor("partial", [S, 64], F32, kind="ExternalOutput")

    from concourse import library_config

    with (
        nc.sbuf_tensor([S, D], F16) as h,
        nc.sbuf_tensor([S, D], F16) as cg,
        nc.sbuf_tensor([S, D], F16) as d,
        nc.sbuf_tensor([S, D], F16) as sq,
        nc.sbuf_tensor([S, 1], F32) as part,
        nc.sbuf_tensor([S, 1], F32) as zt,
        nc.sbuf_tensor([16, S // 16], mybir.dt.int16) as sidx,
        nc.semaphore("sem_h") as sem_h,
        nc.semaphore("sem_g") as sem_g,
        nc.semaphore("sem_zmem") as sem_zmem,
        nc.semaphore("sem_z") as sem_z,
        nc.semaphore("sem_d") as sem_d,
        nc.semaphore("sem_dve") as sem_dve,
        nc.semaphore("sem_prep") as sem_prep,
        nc.semaphore("sem_sdma") as sem_sdma,
        nc.Block() as block,
    ):

        @block.sync
        def _(sync):
            sync.dma_start(out=h[:], in_=h_t[:]).then_inc(sem_h, 16)
            sync.wait_ge(sem_zmem, 1)
            with nc.allow_non_contiguous_dma(reason="128x4B column pre-zero"):
                sync.dma_start(out=o_t[:, :1], in_=zt[:]).then_inc(sem_z, 16)
            sync.wait_ge(sem_sdma, 16)

        @block.gpsimd
        def _(g):
            # critical-path gather first; everything after overlaps the waits
            g.indirect_dma_start(
                out=cg[:],
                out_offset=None,
                in_=c_t[:],
                in_offset=bass.IndirectOffsetOnAxis(ap=y_t[:, :1], axis=0),
            ).then_inc(sem_g, 16)
            # identity scatter indices: idx[p, s] = s*16 + p (InstIota is in
            # the standard library; emit before switching to mlp)
            g.iota(sidx[:], pattern=[[16, S // 16]], base=0, channel_multiplier=1)
            g.load_library(library_config.mlp)
            g.dma_scatter_add(
                o_t[:, :1],
                part[:, :1],
                sidx[:],
                S,
                S,
                1,
                elem_step=64,
                prepare_only=True,
                sem=sem_sdma,
            ).then_inc(sem_prep, 1)
            g.wait_ge(sem_prep, 1)
            g.wait_ge(sem_z, 16)
            g.wait_ge(sem_dve, 1)
            g.trigger_dma(count=1)

        @block.vector
        def _(v):
            v.memset(zt[:], 0.0).then_inc(sem_zmem, 1)
            v.wait_ge(sem_g, 16)
            v.wait_ge(sem_h, 16)
            v.tensor_sub(out=d[:], in0=h[:], in1=cg[:]).then_inc(sem_d, 1)
            v.wait_ge(sem_d, 1)
            v.scalar_tensor_tensor(
                out=sq[:],
                in0=d[:],
                scalar=1.0,
                in1=d[:],
                op0=mybir.AluOpType.mult,
                op1=mybir.AluOpType.mult,
                accum_out=part[:, :1],
            ).then_inc(sem_dve, 1)

        nc.multi_engine_barrier(
            [mybir.EngineType.Pool, mybir.EngineType.DVE, mybir.EngineType.SP]
        )
        for s in (sem_h, sem_g, sem_zmem, sem_z, sem_d, sem_dve, sem_prep, sem_sdma):
            nc.gpsimd.sem_clear(s)

    return nc


def _build_v8():
    """raw2 structure (idx -> SBUF -> indirect gather; walrus requires the
    offset vector in SBUF) with the f16 data path of v6."""
    nc = bass.Bass()
    y_t = nc.dram_tensor("y_idx", [S, 1], mybir.dt.int32, kind="ExternalInput")
    h_t = nc.dram_tensor("h16", [S, D], F16, kind="ExternalInput")
    c_t = nc.dram_tensor("c16", [C, D], F16, kind="ExternalInput")
    o_t = nc.dram_tensor("partial", [S, 1], F32, kind="ExternalOutput")

    with (
        nc.sbuf_tensor([S, 1], mybir.dt.int32) as idx,
        nc.sbuf_tensor([S, D], F16) as h,
        nc.sbuf_tensor([S, D], F16) as cg,
        nc.sbuf_tensor([S, D], F16) as d,
        nc.sbuf_tensor([S, D], F16) as sq,
        nc.sbuf_tensor([S, 1], F32) as part,
        nc.semaphore("sem_idx") as sem_idx,
        nc.semaphore("sem_h") as sem_h,
        nc.semaphore("sem_g") as sem_g,
        nc.semaphore("sem_d") as sem_d,
        nc.semaphore("sem_dve") as sem_dve,
        nc.semaphore("sem_out") as sem_out,
        nc.Block() as block,
    ):

        @block.sync
        def _(sync):
            sync.dma_start(out=idx[:], in_=y_t[:]).then_inc(sem_idx, 16)
            sync.wait_ge(sem_dve, 1)
            sync.dma_start(out=o_t[:], in_=part[:, :1]).then_inc(sem_out, 16)
            sync.wait_ge(sem_out, 16)

        @block.scalar
        def _(sc):
            sc.dma_start(out=h[:], in_=h_t[:]).then_inc(sem_h, 16)

        @block.gpsimd
        def _(g):
            g.wait_ge(sem_idx, 16)
            g.indirect_dma_start(
                out=cg[:],
                out_offset=None,
                in_=c_t[:],
                in_offset=bass.IndirectOffsetOnAxis(ap=idx[:, :1], axis=0),
            ).then_inc(sem_g, 16)

        @block.vector
        def _(v):
            v.wait_ge(sem_g, 16)
            v.wait_ge(sem_h, 16)
            v.tensor_sub(out=d[:], in0=h[:], in1=cg[:]).then_inc(sem_d, 1)
            v.wait_ge(sem_d, 1)
            v.scalar_tensor_tensor(
                out=sq[:],
                in0=d[:],
                scalar=1.0,
                in1=d[:],
                op0=mybir.AluOpType.mult,
                op1=mybir.AluOpType.mult,
                accum_out=part[:, :1],
            ).then_inc(sem_dve, 1)

        nc.multi_engine_barrier(
            [
                mybir.EngineType.Pool,
                mybir.EngineType.Activation,
                mybir.EngineType.DVE,
                mybir.EngineType.SP,
            ]
        )
        for s in (sem_idx, sem_h, sem_g, sem_d, sem_dve, sem_out):
            nc.gpsimd.sem_clear(s)

    return nc


def _build_v9():
    """v8 + the output leaves via PREPARE_ONLY dma_scatter_add + trigger_dma
    (skips the plain out-DMA's HWDGE descgen and DGE launch delay)."""
    from concourse import library_config

    nc = bass.Bass()
    y_t = nc.dram_tensor("y_idx", [S, 1], mybir.dt.int32, kind="ExternalInput")
    h_t = nc.dram_tensor("h16", [S, D], F16, kind="ExternalInput")
    c_t = nc.dram_tensor("c16", [C, D], F16, kind="ExternalInput")
    o_t = nc.dram_tensor("partial", [S, 64], F32, kind="ExternalOutput")

    with (
        nc.sbuf_tensor([S, 1], mybir.dt.int32) as idx,
        nc.sbuf_tensor([S, D], F16) as h,
        nc.sbuf_tensor([S, D], F16) as cg,
        nc.sbuf_tensor([S, D], F16) as d,
        nc.sbuf_tensor([S, D], F16) as sq,
        nc.sbuf_tensor([S, 1], F32) as part,
        nc.sbuf_tensor([S, 1], F32) as zt,
        nc.sbuf_tensor([16, S // 16], mybir.dt.int16) as sidx,
        nc.semaphore("sem_idx") as sem_idx,
        nc.semaphore("sem_h") as sem_h,
        nc.semaphore("sem_g") as sem_g,
        nc.semaphore("sem_zmem") as sem_zmem,
        nc.semaphore("sem_z") as sem_z,
        nc.semaphore("sem_d") as sem_d,
        nc.semaphore("sem_dve") as sem_dve,
        nc.semaphore("sem_prep") as sem_prep,
        nc.semaphore("sem_sdma") as sem_sdma,
        nc.Block() as block,
    ):

        @block.sync
        def _(sync):
            sync.dma_start(out=idx[:], in_=y_t[:]).then_inc(sem_idx, 16)
            sync.wait_ge(sem_zmem, 1)
            with nc.allow_non_contiguous_dma(reason="128x4B column pre-zero"):
                sync.dma_start(out=o_t[:, :1], in_=zt[:]).then_inc(sem_z, 16)
            sync.wait_ge(sem_sdma, 16)

        @block.scalar
        def _(sc):
            sc.dma_start(out=h[:], in_=h_t[:]).then_inc(sem_h, 16)

        @block.gpsimd
        def _(g):
            # identity scatter indices while idx is in flight (standard lib)
            g.iota(sidx[:], pattern=[[16, S // 16]], base=0, channel_multiplier=1)
            g.load_library(library_config.mlp)
            g.wait_ge(sem_idx, 16)
            g.indirect_dma_start(
                out=cg[:],
                out_offset=None,
                in_=c_t[:],
                in_offset=bass.IndirectOffsetOnAxis(ap=idx[:, :1], axis=0),
            ).then_inc(sem_g, 16)
            g.dma_scatter_add(
                o_t[:, :1],
                part[:, :1],
                sidx[:],
                S,
                S,
                1,
                elem_step=64,
                prepare_only=True,
                sem=sem_sdma,
            ).then_inc(sem_prep, 1)
            g.wait_ge(sem_prep, 1)
            g.wait_ge(sem_z, 16)
            g.wait_ge(sem_dve, 1)
            g.trigger_dma(count=1)

        @block.vector
        def _(v):
            v.memset(zt[:], 0.0).then_inc(sem_zmem, 1)
            v.wait_ge(sem_g, 16)
            v.wait_ge(sem_h, 16)
            v.tensor_sub(out=d[:], in0=h[:], in1=cg[:]).then_inc(sem_d, 1)
            v.wait_ge(sem_d, 1)
            v.scalar_tensor_tensor(
                out=sq[:],
                in0=d[:],
                scalar=1.0,
                in1=d[:],
                op0=mybir.AluOpType.mult,
                op1=mybir.AluOpType.mult,
                accum_out=part[:, :1],
            ).then_inc(sem_dve, 1)

        nc.multi_engine_barrier(
            [
                mybir.EngineType.Pool,
                mybir.EngineType.Activation,
                mybir.EngineType.DVE,
                mybir.EngineType.SP,
            ]
        )
        for s in (
            sem_idx,
            sem_h,
            sem_g,
            sem_zmem,
            sem_z,
            sem_d,
            sem_dve,
            sem_prep,
            sem_sdma,
        ):
            nc.gpsimd.sem_clear(s)

    return nc


def _build_v10():
    """v9 + the gather itself is a PREPARE_ONLY dma_gather fired by
    trigger_dma (skips the 650ns DGE->DMA launch delay of the indirect
    copy). Gather indices are int16 in the SWDGE wrapped layout
    [16, S/16] (position i -> partition i%16, column i//16), prepared
    host-side."""
    from concourse import library_config

    nc = bass.Bass()
    y_t = nc.dram_tensor("y16w", [16, S // 16], mybir.dt.int16, kind="ExternalInput")
    h_t = nc.dram_tensor("h16", [S, D], F16, kind="ExternalInput")
    c_t = nc.dram_tensor("c16", [C, D], F16, kind="ExternalInput")
    o_t = nc.dram_tensor("partial", [S, 64], F32, kind="ExternalOutput")

    with (
        nc.sbuf_tensor([16, S // 16], mybir.dt.int16) as idx,
        nc.sbuf_tensor([S, 1, D], F16) as cg,
        nc.sbuf_tensor([S, D], F16) as h,
        nc.sbuf_tensor([S, D], F16) as d,
        nc.sbuf_tensor([S, D], F16) as sq,
        nc.sbuf_tensor([S, 1], F32) as part,
        nc.sbuf_tensor([S, 1], F32) as zt,
        nc.sbuf_tensor([16, S // 16], mybir.dt.int16) as sidx,
        nc.semaphore("sem_idx") as sem_idx,
        nc.semaphore("sem_h") as sem_h,
        nc.semaphore("sem_g") as sem_g,
        nc.semaphore("sem_zmem") as sem_zmem,
        nc.semaphore("sem_z") as sem_z,
        nc.semaphore("sem_gprep") as sem_gprep,
        nc.semaphore("sem_d") as sem_d,
        nc.semaphore("sem_dve") as sem_dve,
        nc.semaphore("sem_prep") as sem_prep,
        nc.semaphore("sem_sdma") as sem_sdma,
        nc.Block() as block,
    ):

        @block.sync
        def _(sync):
            sync.dma_start(out=idx[:], in_=y_t[:]).then_inc(sem_idx, 16)
            sync.wait_ge(sem_zmem, 1)
            with nc.allow_non_contiguous_dma(reason="128x4B column pre-zero"):
                sync.dma_start(out=o_t[:, :1], in_=zt[:]).then_inc(sem_z, 16)
            sync.wait_ge(sem_sdma, 16)

        @block.scalar
        def _(sc):
            sc.dma_start(out=h[:], in_=h_t[:]).then_inc(sem_h, 16)

        @block.gpsimd
        def _(g):
            g.iota(sidx[:], pattern=[[16, S // 16]], base=0, channel_multiplier=1)
            g.load_library(library_config.mlp)
            g.wait_ge(sem_idx, 16)
            g.dma_gather(
                cg[:],
                c_t[:],
                idx[:],
                S,
                S,
                D,
                prepare_only=True,
                sem=sem_g,
            ).then_inc(sem_gprep, 1)
            g.wait_ge(sem_gprep, 1)
            g.trigger_dma(count=1)
            g.dma_scatter_add(
                o_t[:, :1],
                part[:, :1],
                sidx[:],
                S,
                S,
                1,
                elem_step=64,
                prepare_only=True,
                sem=sem_sdma,
            ).then_inc(sem_prep, 1)
            g.wait_ge(sem_prep, 1)
            g.wait_ge(sem_z, 16)
            g.wait_ge(sem_dve, 1)
            g.trigger_dma(count=1)

        @block.vector
        def _(v):
            v.memset(zt[:], 0.0).then_inc(sem_zmem, 1)
            v.wait_ge(sem_g, 16)
            v.wait_ge(sem_h, 16)
            v.tensor_sub(
                out=d[:], in0=h[:], in1=cg[:, 0, :]
            ).then_inc(sem_d, 1)
            v.wait_ge(sem_d, 1)
            v.scalar_tensor_tensor(
                out=sq[:],
                in0=d[:],
                scalar=1.0,
                in1=d[:],
                op0=mybir.AluOpType.mult,
                op1=mybir.AluOpType.mult,
                accum_out=part[:, :1],
            ).then_inc(sem_dve, 1)

        nc.multi_engine_barrier(
            [
                mybir.EngineType.Pool,
                mybir.EngineType.Activation,
                mybir.EngineType.DVE,
                mybir.EngineType.SP,
            ]
        )
        for s in (
            sem_idx,
            sem_h,
            sem_g,
            sem_zmem,
            sem_z,
            sem_gprep,
            sem_d,
            sem_dve,
            sem_prep,
            sem_sdma,
        ):
            nc.gpsimd.sem_clear(s)

    return nc


def _strip_preamble(nc):
    """Remove the framework's const-table memsets plus the entry AND exit
    all-engine barriers (with their fencing drains). Nothing in this kernel
    reads the const tiles, and the kernel's own semaphore protocol fully
    orders the engines, so the barriers only add ~1.2us of latency."""
    removed = 0
    for blk in nc.m.functions[0].blocks:
        keep = []
        for ins in blk.instructions:
            txt = ins.concise()
            drop = "barrier_Pool_Activation_PE_DVE_SP" in txt or (
                "Memset" in txt and "const-" in txt
            )
            if drop:
                removed += 1
            else:
                keep.append(ins)
        blk.instructions = keep
    return removed


def _build_v13():
    """v8 + (a) no DVE self-semaphore between the subtract and the
    square-accumulate (same-engine in-order RAW), (b) preamble stripped of
    the const-table memsets and entry barrier, (c) lean SP-side epilogue."""
    nc = bass.Bass()
    y_t = nc.dram_tensor("y_idx", [S, 1], mybir.dt.int32, kind="ExternalInput")
    h_t = nc.dram_tensor("h16", [S, D], F16, kind="ExternalInput")
    c_t = nc.dram_tensor("c16", [C, D], F16, kind="ExternalInput")
    o_t = nc.dram_tensor("partial", [S, 1], F32, kind="ExternalOutput")

    with (
        nc.sbuf_tensor([S, 1], mybir.dt.int32) as idx,
        nc.sbuf_tensor([S, D], F16) as h,
        nc.sbuf_tensor([S, D], F16) as cg,
        nc.sbuf_tensor([S, D], F16) as d,
        nc.sbuf_tensor([S, D], F16) as sq,
        nc.sbuf_tensor([S, 1], F32) as part,
        nc.semaphore("sem_idx") as sem_idx,
        nc.semaphore("sem_h") as sem_h,
        nc.semaphore("sem_g") as sem_g,
        nc.semaphore("sem_dve") as sem_dve,
        nc.semaphore("sem_out") as sem_out,
        nc.Block() as block,
    ):

        @block.sync
        def _(sync):
            sync.dma_start(out=idx[:], in_=y_t[:]).then_inc(sem_idx, 16)
            # out-DMA waits on sem_dve at its own SEQ stage (attached wait)
            sync.dma_start(out=o_t[:], in_=part[:, :1])._wait_ge(
                sem_dve, 1
            ).then_inc(sem_out, 16)
            # non-terminal sems: every consumer strictly precedes sem_dve,
            # so clearing here overlaps the out-DMA's ~2.2us latency
            for s in (sem_idx, sem_h, sem_g, sem_dve):
                sync.sem_clear(s)
            sync.wait_ge(sem_out, 16)
            sync.sem_clear(sem_out)

        @block.scalar
        def _(sc):
            sc.dma_start(out=h[:], in_=h_t[:]).then_inc(sem_h, 16)

        @block.gpsimd
        def _(g):
            g.indirect_dma_start(
                out=cg[:],
                out_offset=None,
                in_=c_t[:],
                in_offset=bass.IndirectOffsetOnAxis(ap=idx[:, :1], axis=0),
            )._wait_ge(sem_idx, 16).then_inc(sem_g, 16)

        @block.vector
        def _(v):
            v.wait_ge(sem_h, 16)  # h lands early; this clears while idle
            v.tensor_sub(out=d[:], in0=h[:], in1=cg[:])._wait_ge(sem_g, 16)
            v.scalar_tensor_tensor(
                out=sq[:],
                in0=d[:],
                scalar=1.0,
                in1=d[:],
                op0=mybir.AluOpType.mult,
                op1=mybir.AluOpType.mult,
                accum_out=part[:, :1],
            ).then_inc(sem_dve, 1)

    _strip_preamble(nc)
    return nc


DAUG = D + 128  # augmented center row: [c_0..c_511, |c|^2, pad...] f16
DAUG2 = D + 8  # v15: slim augmented row (indirect DMA has no 256B rule)


def _build_v15():
    """v14 with the proven [S,1] idx layout and a slim augmented row
    (DAUG2 = 520 f16 = 1040B/row; indirect DMA has no 256B stride rule)."""
    nc = bass.Bass()
    y_t = nc.dram_tensor("y_idx", [S, 1], mybir.dt.int32, kind="ExternalInput")
    h_t = nc.dram_tensor("h16", [S, D], F16, kind="ExternalInput")
    c_t = nc.dram_tensor("c16", [C, DAUG2], F16, kind="ExternalInput")
    o_t = nc.dram_tensor("partial", [S, 4], F32, kind="ExternalOutput")

    with (
        nc.sbuf_tensor([S, 1], mybir.dt.int32) as idx,
        nc.sbuf_tensor([S, D], F16) as h,
        nc.sbuf_tensor([S, DAUG2], F16) as cg,
        nc.sbuf_tensor([S, D], F16) as sqa,
        nc.sbuf_tensor([S, D], F16) as sqb,
        nc.sbuf_tensor([S, 4], F32) as part,
        nc.semaphore("sem_idx") as sem_idx,
        nc.semaphore("sem_h") as sem_h,
        nc.semaphore("sem_g") as sem_g,
        nc.semaphore("sem_dve") as sem_dve,
        nc.semaphore("sem_out") as sem_out,
        nc.Block() as block,
    ):

        @block.sync
        def _(sync):
            sync.dma_start(out=idx[:], in_=y_t[:]).then_inc(sem_idx, 16)
            sync.dma_start(out=o_t[:], in_=part[:, :4])._wait_ge(
                sem_dve, 1
            ).then_inc(sem_out, 16)
            for s in (sem_idx, sem_h, sem_g, sem_dve):
                sync.sem_clear(s)
            sync.wait_ge(sem_out, 16)
            sync.sem_clear(sem_out)

        @block.scalar
        def _(sc):
            sc.dma_start(out=h[:], in_=h_t[:]).then_inc(sem_h, 16)

        @block.gpsimd
        def _(g):
            g.indirect_dma_start(
                out=cg[:],
                out_offset=None,
                in_=c_t[:],
                in_offset=bass.IndirectOffsetOnAxis(ap=idx[:, :1], axis=0),
            )._wait_ge(sem_idx, 16).then_inc(sem_g, 16)

        @block.vector
        def _(v):
            v.scalar_tensor_tensor(
                out=sqa[:],
                in0=h[:],
                scalar=1.0,
                in1=h[:],
                op0=mybir.AluOpType.mult,
                op1=mybir.AluOpType.mult,
                accum_out=part[:, 0:1],
            )._wait_ge(sem_h, 16)
            v.scalar_tensor_tensor(
                out=sqb[:],
                in0=h[:],
                scalar=-2.0,
                in1=cg[:, :D],
                op0=mybir.AluOpType.mult,
                op1=mybir.AluOpType.mult,
                accum_out=part[:, 1:2],
            )._wait_ge(sem_g, 16)
            v.tensor_copy(out=part[:, 2:3], in_=cg[:, D : D + 1]).then_inc(
                sem_dve, 1
            )

    _strip_preamble(nc)
    return nc


def _build_v14():
    """v13 + the ||h-c||^2 = ||h||^2 - 2<h,c> + ||c||^2 expansion:
    the centers table is host-augmented with ||c||^2 per row, so after the
    gather the DVE only runs one tiny copy (the gathered ||c||^2 column)
    plus one STT (-2<h,c>); ||h||^2 is computed early, while the gather is
    still in flight. Partials land in three columns summed on the host."""
    nc = bass.Bass()
    y_t = nc.dram_tensor("y_idx", [1, S], mybir.dt.int32, kind="ExternalInput")
    h_t = nc.dram_tensor("h16", [S, D], F16, kind="ExternalInput")
    c_t = nc.dram_tensor("c16", [C, DAUG], F16, kind="ExternalInput")
    o_t = nc.dram_tensor("partial", [S, 4], F32, kind="ExternalOutput")

    with (
        nc.sbuf_tensor([1, S], mybir.dt.int32) as idx,
        nc.sbuf_tensor([S, D], F16) as h,
        nc.sbuf_tensor([S, DAUG], F16) as cg,
        nc.sbuf_tensor([S, D], F16) as sqa,
        nc.sbuf_tensor([S, D], F16) as sqb,
        nc.sbuf_tensor([S, 4], F32) as part,
        nc.semaphore("sem_idx") as sem_idx,
        nc.semaphore("sem_h") as sem_h,
        nc.semaphore("sem_g") as sem_g,
        nc.semaphore("sem_dve") as sem_dve,
        nc.semaphore("sem_out") as sem_out,
        nc.Block() as block,
    ):

        @block.sync
        def _(sync):
            sync.dma_start(out=idx[:], in_=y_t[:]).then_inc(sem_idx, 16)
            sync.dma_start(out=o_t[:], in_=part[:, :4])._wait_ge(
                sem_dve, 1
            ).then_inc(sem_out, 16)
            for s in (sem_idx, sem_h, sem_g, sem_dve):
                sync.sem_clear(s)
            sync.wait_ge(sem_out, 16)
            sync.sem_clear(sem_out)

        @block.scalar
        def _(sc):
            sc.dma_start(out=h[:], in_=h_t[:]).then_inc(sem_h, 16)

        @block.gpsimd
        def _(g):
            g.indirect_dma_start(
                out=cg[:],
                out_offset=None,
                in_=c_t[:],
                in_offset=bass.IndirectOffsetOnAxis(ap=idx[:1, :S], axis=0),
            )._wait_ge(sem_idx, 16).then_inc(sem_g, 16)

        @block.vector
        def _(v):
            # ||h||^2 while the gather is still in flight (free)
            v.scalar_tensor_tensor(
                out=sqa[:],
                in0=h[:],
                scalar=1.0,
                in1=h[:],
                op0=mybir.AluOpType.mult,
                op1=mybir.AluOpType.mult,
                accum_out=part[:, 0:1],
            )._wait_ge(sem_h, 16)
            # gathered ||c||^2 column -> f32 partial
            v.tensor_copy(out=part[:, 2:3], in_=cg[:, D : D + 1])._wait_ge(
                sem_g, 16
            )
            # -2<h,c>
            v.scalar_tensor_tensor(
                out=sqb[:],
                in0=h[:],
                scalar=-2.0,
                in1=cg[:, :D],
                op0=mybir.AluOpType.mult,
                op1=mybir.AluOpType.mult,
                accum_out=part[:, 1:2],
            ).then_inc(sem_dve, 1)

    _strip_preamble(nc)
    return nc


def _build(variant=VARIANT):
    if variant == "raw":
        return _build_raw()
    if variant == "raw2":
        return _build_raw2()
    if variant == "raw3":
        return _build_raw3()
    if variant == "raw4":
        return _build_raw4(2)
    if variant == "raw5":
        return _build_raw5()
    if variant == "raw4x4":
        return _build_raw4(4)
    if variant == "v6":
        return _build_v6()
    if variant == "v7":
        return _build_v7()
    if variant == "v8":
        return _build_v8()
    if variant == "v9":
        return _build_v9()
    if variant == "v10":
        return _build_v10()
    if variant == "v13":
        return _build_v13()
    if variant == "v14":
        return _build_v14()
    if variant == "v15":
        return _build_v15()
    nc = bass.Bass()
    y_t = nc.dram_tensor("y_idx", [S, 1], mybir.dt.int32, kind="ExternalInput")
    h_t = nc.dram_tensor("hidden_shard", [S, D], F32, kind="ExternalInput")
    c_t = nc.dram_tensor("neg_centers", [C, D], F32, kind="ExternalInput")
    o_t = nc.dram_tensor("partial", [S, 1], F32, kind="ExternalOutput")

    with tile.TileContext(nc) as tc:
        with tc.tile_pool(name="p", bufs=1) as pool:
            idx = pool.tile([S, 1], mybir.dt.int32)
            nc.sync.dma_start(out=idx[:], in_=y_t[:])

            t = pool.tile([S, D], F32)
            nc.sync.dma_start(out=t[:], in_=h_t[:])

            if variant == "fused":
                # t := (-centers[y]) + t  (inline CCE add during the gather)
                nc.gpsimd.indirect_dma_start(
                    out=t[:],
                    out_offset=None,
                    in_=c_t[:],
                    in_offset=bass.IndirectOffsetOnAxis(ap=idx[:, :1], axis=0),
                    compute_op=mybir.AluOpType.add,
                )
                d = t
            else:
                cg = pool.tile([S, D], F32)
                nc.gpsimd.indirect_dma_start(
                    out=cg[:],
                    out_offset=None,
                    in_=c_t[:],
                    in_offset=bass.IndirectOffsetOnAxis(ap=idx[:, :1], axis=0),
                )
                # copy h through DVE so the subtract has a single cross-engine
                # wait (this target allows one sync wait per compute inst)
                hc = pool.tile([S, D], F32)
                nc.vector.tensor_copy(out=hc[:], in_=t[:])
                d = pool.tile([S, D], F32)
                # d = cg - hc = (-c) - h ... sign irrelevant after squaring;
                # use add to get (-c) + h = h - c anyway
                nc.vector.tensor_add(out=d[:], in0=hc[:], in1=cg[:])

            sq = pool.tile([S, D], F32)
            part = pool.tile([S, 1], F32)
            nc.vector.scalar_tensor_tensor(
                out=sq[:],
                in0=d[:],
                scalar=SCALE,
                in1=d[:],
                op0=mybir.AluOpType.mult,
                op1=mybir.AluOpType.mult,
                accum_out=part[:, :1],
            )
            nc.sync.dma_start(out=o_t[:], in_=part[:, :1])
    return nc


_NC = None


def _get_nc():
    global _NC
    if _NC is None:
        _NC = _build()
    return _NC


def _in_maps(y, hidden, centers):
    y = np.ascontiguousarray(np.asarray(y).astype(np.int32).reshape(B, 1))
    if VARIANT in ("v14", "v15"):
        h16 = np.ascontiguousarray(np.asarray(hidden).astype(np.float16))
        c64 = np.asarray(centers, dtype=np.float64)
        daug = DAUG if VARIANT == "v14" else DAUG2
        caug = np.zeros((C, daug), dtype=np.float16)
        caug[:, :D] = c64.astype(np.float16)
        caug[:, D] = (c64 * c64).sum(axis=1).astype(np.float16)
        caug = np.ascontiguousarray(caug)
        yv = (
            (lambda a: a.reshape(1, S)) if VARIANT == "v14" else (lambda a: a)
        )
        return [
            {
                "y_idx": yv(y[i * S : (i + 1) * S]),
                "h16": h16[i * S : (i + 1) * S],
                "c16": caug,
            }
            for i in range(N_CORES)
        ]
    if VARIANT in ("v6", "v7", "v8", "v9", "v10", "v13"):
        h16 = np.ascontiguousarray(np.asarray(hidden).astype(np.float16))
        c16 = np.ascontiguousarray(np.asarray(centers).astype(np.float16))
        if VARIANT == "v10":
            # SWDGE wrapped idx layout: position i -> [i % 16, i // 16]
            y16 = y.reshape(N_CORES, S).astype(np.int16)
            y16w = np.ascontiguousarray(
                y16.reshape(N_CORES, S // 16, 16).transpose(0, 2, 1)
            )
            return [
                {
                    "y16w": y16w[i],
                    "h16": h16[i * S : (i + 1) * S],
                    "c16": c16,
                }
                for i in range(N_CORES)
            ]
        return [
            {
                "y_idx": y[i * S : (i + 1) * S],
                "h16": h16[i * S : (i + 1) * S],
                "c16": c16,
            }
            for i in range(N_CORES)
        ]
    hidden = np.ascontiguousarray(np.asarray(hidden, dtype=np.float32))
    negc = np.ascontiguousarray(-np.asarray(centers, dtype=np.float32))
    return [
        {
            "y_idx": y[i * S : (i + 1) * S],
            "hidden_shard": hidden[i * S : (i + 1) * S],
            "neg_centers": negc,
        }
        for i in range(N_CORES)
    ]


def kernel(y, hidden, centers, _trace=False, _trace_kwargs=None):
    res = run_bass_kernel_spmd(
        _get_nc(),
        _in_maps(y, hidden, centers),
        core_ids=list(range(N_CORES)),
        trace=_trace,
        **(_trace_kwargs or {}),
    )
    total = np.float64(0.0)
    for r in res.results:
        p = r["partial"]
        if VARIANT in ("v14", "v15"):  # three-term expansion in cols 0..2
            p = p[:, :3]
        elif p.ndim == 2 and p.shape[1] > 1:  # v7: partial sits in column 0
            p = p[:, 0]
        total += np.float64(p.sum(dtype=np.float64))
    if VARIANT in ("v6", "v7", "v8", "v9", "v10", "v13", "v14", "v15"):
        total *= SCALE  # 0.5/B applied on host (kernel accumulates raw sum-sq)
    out = np.float32(total)
    if _trace:
        return out, res
    return out

